# revision 34
# baseline (speedup 1.0000x reference)
"""Trainium2 Bass kernel for nn_Attention_layer_67877663146058.

Computes attn = softmax((x @ Wq.T) @ (x @ Wk.T)^T * hd**-0.5)
for x [8, 1024, 768], W_qkv [2304, 768] -> out [8, 12, 1024, 1024] fp32.
The V third of W_qkv never reaches the output and is not loaded.

Sharding: batch-parallel across the 8 NeuronCores (core b handles batch b,
all 12 heads).

Design notes (evolved from trace analysis across many iterations):
- The kernel is ACT(exp)-paced: 96 x [128,1024] exp tiles per core at the
  1x dtype-independent ACT rate (~1.03ns/elem + ~183ns/instr) is ~97us;
  everything else is scheduled to hide under it. DVE (~101us: identity
  row-sums at 1x, proj casts, recips) and PE (~103us incl. exposed
  drains) are near-co-bound; GpSimd carries most normalize multiplies.
- Output is fp16 scaled by 1024 (smallest softmax entries ~2e-6 would be
  fp16 subnormals); the host upcast multiplies by the exact 1/1024.
- Row sums: A-slices (3,6,7) use the free ACT accumulator (standalone
  [128,1024] exps + 182ns READ_ACCUMULATOR); G-runs tile cleanly as
  6144=4x1536 / 4096=2x1536+1024 spans with DVE tensor_scalar+accum sums.
  The last f-tile is A-heavy (3..7): its DVE would otherwise saturate on
  sums and delay the final normalize+DMA drain. Its normalize pairs are
  split GpSimd/DVE so they run concurrently, and the final slice's DMA
  is split per-1024 so the last transfer is small. Drain after the last
  exp is ~6.5us including ~3us of fixed end-of-NEFF overhead.
- Input DMA: one HWDGE ring only sustains ~210GB/s on HBM reads, so the
  fill-critical wt0+x (1.9MB) is split across the Sync AND Scalar rings
  (~350GB/s combined); each ring drains FIFO so the remaining weights
  (wt1 f-tile 1 / wt2 rest, needed ~17us+ in) queue strictly behind x
  and cannot round-robin bandwidth away from it.
- Fill: the f-tile-0 projection runs e-tile-outer over K0/K1/Q0 so its
  matmuls overlap the x chunk arrivals; Q1 (first needed by slice 4) is
  deferred into the interleave stream. The tail x chunk is quarter-major
  so the K0->K1->Q0 casts cascade early (K0 on the otherwise-idle ACT,
  rest on DVE). A dependency-free exp at t=0 preloads the ACT spline
  table; 9 dummy matmuls bridge PE HAM (a >3.4us PE-idle window would
  re-throttle the clock 2.4->1.2GHz and double the fill matmul time).
- fi+1's projection threads into fi's span stream as deprioritized
  pulled steps (tc.high_priority(-25)) so the scheduler prefers the
  imminent span's score matmuls; an in-order-PE-queue stall on a proj
  matmul whose PSUM slot awaits its DVE cast is the main residual ACT
  gap mechanism (~6-8us across f0-f2).
- Score matmuls put the two heads of an f-tile in PE row groups 0-63 /
  64-127 (tile_position) and alternate row groups within a span so
  adjacent matmuls can overlap in the array.
- PSUM: 2x [128,512] proj slots + 2x [128,1536] score slots = 8 banks.
"""

import numpy as np
from contextlib import ExitStack

import concourse.bacc as bacc
import concourse.mybir as mybir
import concourse.tile as tile

# bass_utils imports antenv.axon_hooks when BASS_TRACE is set in the
# environment; some images ship an antenv stub without that module. Register
# a no-op fallback so tracing degrades gracefully instead of crashing.
try:
    from antenv.axon_hooks import get_axon_ntff_profile_hook as _g  # noqa: F401
except Exception:
    import sys as _sys
    import types as _types

    _m = _types.ModuleType("antenv.axon_hooks")
    _state = {"h": None}
    _m.set_axon_ntff_profile_hook = lambda h: _state.__setitem__("h", h)
    _m.get_axon_ntff_profile_hook = lambda: _state["h"]
    _sys.modules["antenv.axon_hooks"] = _m
    try:
        import antenv as _antenv

        _antenv.axon_hooks = _m
    except Exception:
        pass

from concourse.bass_utils import run_bass_kernel_spmd

B = 8          # batches == cores
N = 1024       # tokens
E = 768        # embed dim
H = 12         # heads
HD = 64        # head dim
FT = 6         # f-tiles (2 heads per f-tile)
ET = E // 128  # 6 e-tiles
SPAN = 1536    # G-region exp span (one PSUM score slot, 3 banks)
SCALE = HD ** -0.5
OUT_SCALE = 1024.0   # fp16 output holds out*1024 to avoid subnormals

# qb slices whose sums come from the ACT accumulator. (3,6,7) tiles the
# G region as 6144=4x1536 + 4096=2x1536+1024 (no 512 fragments) and ends
# every f-tile with two self-contained A-slices, so the last f-tile's
# drain after the final exp is just recip+mult+DMA. The last f-tile uses
# more A-slices: its DVE would otherwise saturate on sums and delay the
# final normalize+DMA chain.
A_SLICES = (3, 6, 7)
A_SLICES_LAST = (3, 4, 5, 6, 7)
# pulled projection steps are deprioritized by this many emission slots
# so the scheduler runs the imminent span's score matmuls first
PULL_DEPRIO = 25
# steady-f-tile slices whose normalize mults run on DVE (rest GpSimd);
# the last f-tile splits every slice's pair across both engines instead
DVE_MULT_STEADY = (3,)

_cache = {}


def _build():
    f32 = mybir.dt.float32
    f16 = mybir.dt.float16
    mult = mybir.AluOpType.mult
    add = mybir.AluOpType.add
    Exp = mybir.ActivationFunctionType.Exp
    nc = bacc.Bacc("TRN2", debug=False, num_devices=B)

    # inputs are packed partition-major on the host so each DMA moves
    # multi-KB contiguous runs per partition (128 fat descriptors/DMA)
    xP_d = nc.dram_tensor("xP", [128, ET * N], f16, kind="ExternalInput")
    wA_d = nc.dram_tensor("wA", [128, ET * 256], f16, kind="ExternalInput")
    wB1_d = nc.dram_tensor("wB1", [128, ET * 256], f16, kind="ExternalInput")
    wB2_d = nc.dram_tensor("wB2", [128, ET * 1024], f16, kind="ExternalInput")
    out_d = nc.dram_tensor("out", [FT * 8, 128, 2048], f16, kind="ExternalOutput")

    xP_src = xP_d.ap().rearrange("p (t n) -> p t n", t=ET)      # [128,6,1024]
    wA_src = wA_d.ap().rearrange("p (t c) -> p t c", t=ET)      # [128,6,256]
    wB1_src = wB1_d.ap().rearrange("p (t c) -> p t c", t=ET)    # [128,6,256]
    wB2_src = wB2_d.ap().rearrange("p (t c) -> p t c", t=ET)    # [128,6,1024]
    out_ap = out_d.ap()

    with ExitStack() as ctx:
        tc = ctx.enter_context(tile.TileContext(nc))
        statics = ctx.enter_context(tc.tile_pool(name="statics", bufs=1))
        ypool = ctx.enter_context(tc.tile_pool(name="ypool", bufs=2))
        spool = ctx.enter_context(tc.tile_pool(name="spool", bufs=2))
        psum = ctx.enter_context(tc.tile_pool(name="psum", bufs=2, space="PSUM"))

        xt = statics.tile([128, ET, N], f16, tag="xt", name="xt")
        # W columns split into three contiguous tiles (f-tile 0 / 1 / rest)
        # so each input DMA moves multi-KB runs per partition and f-tile 1's
        # weights (needed first for the interleaved projection) land with
        # their own completion instead of behind the whole remainder
        wt0 = statics.tile([128, ET, 256], f16, tag="wt0", name="wt0")
        wt1 = statics.tile([128, ET, 256], f16, tag="wt1", name="wt1")
        wt2 = statics.tile([128, ET, 1024], f16, tag="wt2", name="wt2")
        # qkt[:, fi, 0, :] = K^T of f-tile fi, qkt[:, fi, 1, :] = Q^T
        qkt = statics.tile([128, FT, 2, N], f16, tag="qkt", name="qkt")

        def w_block(fi, kq, ei):
            if fi == 0:
                return wt0[:, ei, kq * 128:(kq + 1) * 128]
            if fi == 1:
                return wt1[:, ei, kq * 128:(kq + 1) * 128]
            c = (fi - 2) * 256 + kq * 128
            return wt2[:, ei, c:c + 128]

        # ACT table preload: dependency-free exp at t=0 pulls the ~2.7us
        # ACT_TABLE_LOAD off the critical path of the first real exp.
        warm = spool.tile([128, 1], f32, tag="warm", name="warm")
        nc.vector.memset(warm, 0.0)

        # PE HAM warm-up weights
        wl = statics.tile([128, 128], f16, tag="wl", name="wl")
        wr = statics.tile([128, 512], f16, tag="wr", name="wr")
        nc.vector.memset(wl, 0.0)
        nc.vector.memset(wr, 0.0)

        # Input loads. A single HWDGE ring only sustains ~210 GB/s on HBM
        # reads, so the fill-critical wt0+x (1.9MB) is split across BOTH
        # rings (Sync + Scalar, ~350 GB/s combined). Each ring drains its
        # entries FIFO, so the remaining weights (needed only ~17us in)
        # sit strictly BEHIND x on the Scalar ring and cannot round-robin
        # bandwidth away from it.
        nc.sync.dma_start(wt0, wA_src)
        nc.scalar.dma_start(xt[:, 0:2, :], xP_src[:, 0:2, :])
        nc.sync.dma_start(xt[:, 2:4, :], xP_src[:, 2:4, :])
        nc.scalar.dma_start(xt[:, 4:6, :], xP_src[:, 4:6, :])
        nc.scalar.dma_start(wt1, wB1_src)
        nc.scalar.dma_start(wt2, wB2_src)

        nc.scalar.activation(warm, warm, Exp)

        # PE HAM warm-up: dummy matmuls keep the PE busy (and its clock
        # gate at 2.4 GHz) until the first x chunk lands; a >3.4us PE-idle
        # window would re-throttle it to 1.2 GHz and double the fill
        # projection's matmul time. Heavily deprioritized so the scheduler
        # only slots them where the PE would otherwise idle — they must
        # never delay a real matmul. Parks in a "ps" slot.
        pw = psum.tile([128, SPAN], f32, tag="ps", name="pw")
        for _ in range(9):
            nc.tensor.matmul(pw[:, 0:512], lhsT=wl, rhs=wr, start=True,
                             stop=True, skip_group_check=True)

        QUARTERS = [(0, 0), (0, 1), (1, 0), (1, 1)]  # K halves first

        # f-tile 0 projection, e-tile outer: the K0/K1/Q0 quarter
        # accumulations proceed as each x chunk lands instead of waiting
        # for all of x. K quarters borrow the (idle) score slots so their
        # casts free them for the first spans; Q0 sits in a proj slot.
        # Q1 (first needed by slice 4, several spans in) is deferred into
        # f-tile 0's interleave stream entirely.
        FILL_Q = [(0, 0, "ps"), (0, 1, "ps"), (1, 0, "pp")]
        f0_tiles = []
        for qi, (kq, half, tag) in enumerate(FILL_Q):
            shape = [128, SPAN] if tag == "ps" else [128, 512]
            f0_tiles.append(psum.tile(shape, f32, tag=tag, name=f"f0q{qi}"))

        def f0_mm(qi, ei):
            kq, half, _ = FILL_Q[qi]
            nc.tensor.matmul(
                f0_tiles[qi][:, 0:512],
                lhsT=w_block(0, kq, ei),
                rhs=xt[:, ei, half * 512:(half + 1) * 512],
                start=(ei == 0),
                stop=(ei == ET - 1),
                skip_group_check=True,
            )

        for ei in range(4):
            for qi in range(3):
                f0_mm(qi, ei)
        # last x chunk: quarter-major so K0 finishes first and its cast
        # overlaps the remaining matmuls
        for qi in range(3):
            for ei in (4, 5):
                f0_mm(qi, ei)
        # Casts needed before the first span: K both halves + the first
        # 256 Q columns (the first span's lhsT only reads qb0/qb1). Split
        # across the otherwise-idle ACT engine and DVE.
        nc.scalar.copy(qkt[:, 0, 0, 0:512], f0_tiles[0][:, 0:512])
        nc.vector.tensor_copy(qkt[:, 0, 0, 512:1024], f0_tiles[1][:, 0:512])
        nc.vector.tensor_copy(qkt[:, 0, 1, 0:256], f0_tiles[2][:, 0:256])
        nc.vector.tensor_copy(qkt[:, 0, 1, 256:512], f0_tiles[2][:, 256:512])

        def f0_q1_steps():
            holder = []
            for ei in range(ET):
                def mm_step(ei=ei, holder=holder):
                    if ei == 0:
                        holder.append(psum.tile(
                            [128, 512], f32, tag="pp", name="f0q3"))
                    nc.tensor.matmul(
                        holder[0],
                        lhsT=w_block(0, 1, ei),
                        rhs=xt[:, ei, 512:1024],
                        start=(ei == 0),
                        stop=(ei == ET - 1),
                        skip_group_check=True,
                    )
                yield mm_step

            def cast_step(holder=holder):
                nc.vector.tensor_copy(qkt[:, 0, 1, 512:1024], holder[0])
            yield cast_step

        def proj_steps(fi):
            # The projection for f-tile fi as a list of single-instruction
            # closures (6 accumulating matmuls + 1 evacuation cast per
            # quarter). Threading these one or two at a time between score
            # spans keeps the PE from blocking the ACT-paced span stream
            # (accumulation groups don't need to be contiguous in the PE
            # program; they only own their PSUM bank).
            steps = []
            for kq, half in QUARTERS:
                holder = []
                for ei in range(ET):
                    def mm_step(kq=kq, half=half, ei=ei, holder=holder):
                        if ei == 0:
                            holder.append(psum.tile(
                                [128, 512], f32, tag="pp",
                                name=f"pp{fi}_{kq}_{half}",
                            ))
                        nc.tensor.matmul(
                            holder[0],
                            lhsT=w_block(fi, kq, ei),
                            rhs=xt[:, ei, half * 512:(half + 1) * 512],
                            start=(ei == 0),
                            stop=(ei == ET - 1),
                            skip_group_check=True,
                        )
                    steps.append(mm_step)

                def cast_step(kq=kq, half=half, holder=holder):
                    nc.vector.tensor_copy(
                        qkt[:, fi, kq, half * 512:(half + 1) * 512],
                        holder[0],
                    )
                steps.append(cast_step)
            return steps

        def score_mm(ps, off, fi, qb, hh, nh):
            lo = 64 * hh
            nc.tensor.matmul(
                ps[:, off:off + 512],
                lhsT=qkt[lo:lo + 64, fi, 1, qb * 128:(qb + 1) * 128],
                rhs=qkt[lo:lo + 64, fi, 0, nh * 512:(nh + 1) * 512],
                start=True,
                stop=True,
                tile_position=(lo, 0),
            )

        def emit_attn(fi, interleave):
            # Slot s of y holds qb s (identity layout). A-slices (ACT
            # accumulator, self-contained 1024-col exps) are spaced between
            # span-aligned G-runs; the G-region generates more DVE work per
            # ACT span than ACT consumes (identity sums + recip + casts), so
            # A-slices let the DVE queue drain.
            a_last = fi == FT - 1
            a_set = A_SLICES_LAST if a_last else A_SLICES
            segs = []
            run = []
            for s in range(8):
                if s in a_set:
                    if run:
                        segs.append(("G", run))
                        run = []
                    segs.append(("A", [s]))
                else:
                    run.append(s)
            if run:
                segs.append(("G", run))

            y = ypool.tile([128, 16 * N], f16, tag="y", name=f"y{fi}")
            sums = spool.tile([128, 16], f32, tag="sums", name=f"sm{fi}")
            rec = spool.tile([128, 16], f32, tag="rec", name=f"rc{fi}")
            pending = list(interleave)

            def pull(k):
                # deprioritized: the pulled proj matmuls must not crowd
                # ahead of the next span's score matmuls in the PE queue
                with tc.high_priority(-PULL_DEPRIO):
                    for fn in pending[:k]:
                        fn()
                del pending[:k]

            def tile_sum(tt):
                yt = y[:, tt * N:(tt + 1) * N]
                nc.vector.tensor_scalar(
                    yt, yt, 1.0, 0.0, mult, add, accum_out=sums[:, tt:tt + 1],
                )

            def finish_slice(s):
                pull(1)
                t0 = 2 * s
                nc.vector.reciprocal(rec[:, t0:t0 + 2], sums[:, t0:t0 + 2])
                split_dma = a_last and s == 7
                for tt in (t0, t0 + 1):
                    yt = y[:, tt * N:(tt + 1) * N]
                    if a_last:
                        # both engines per slice: the pair runs concurrently,
                        # shortening the drain after the last exp
                        eng = nc.gpsimd if tt == t0 else nc.vector
                    else:
                        eng = nc.vector if s in DVE_MULT_STEADY else nc.gpsimd
                    eng.tensor_scalar(yt, yt, rec[:, tt:tt + 1], OUT_SCALE,
                                      mult, mult)
                    if split_dma:
                        nc.sync.dma_start(
                            out_ap[fi * 8 + s][:, (tt - t0) * N:(tt - t0 + 1) * N],
                            yt,
                        )
                if not split_dma:
                    nc.sync.dma_start(
                        out_ap[fi * 8 + s], y[:, s * 2048:(s + 1) * 2048]
                    )

            # Pulled proj steps land in the PE queue between the emitting
            # span's matmuls and the NEXT span's; inside an A-slice they
            # would delay the following G-span's matmuls past its exp and
            # open an ACT gap, so A-slices don't pull (except f-tile 0,
            # which has more steps to thread).
            a_pull = 2 if fi == 0 else 0
            g_pull = 2 if fi == 0 else 3
            for kind, qbs in segs:
                if kind == "A":
                    s = qbs[0]
                    for tt in (2 * s, 2 * s + 1):
                        hh = tt % 2
                        ps = psum.tile([128, SPAN], f32, tag="ps",
                                       name=f"psA{fi}_{tt}")
                        for nh in range(2):
                            score_mm(ps, nh * 512, fi, s, hh, nh)
                        nc.scalar.activation(
                            y[:, tt * N:(tt + 1) * N], ps[:, 0:N], Exp,
                            scale=SCALE, accum_out=sums[:, tt:tt + 1],
                        )
                        pull(a_pull)
                    finish_slice(s)
                else:
                    run0 = qbs[0] * 2048
                    run1 = run0 + len(qbs) * 2048
                    done = run0
                    for c0 in range(run0, run1, SPAN):
                        L = min(SPAN, run1 - c0)
                        ps = psum.tile([128, SPAN], f32, tag="ps",
                                       name=f"psG{fi}_{c0}")
                        # order chunks so PE row groups (hh) alternate:
                        # adjacent matmuls then run concurrently in the
                        # 128x128 array.
                        chunks = []
                        for off in range(0, L, 512):
                            g = c0 + off
                            chunks.append(
                                (off, g // 2048, (g // 1024) % 2, (g // 512) % 2))
                        h0 = [c for c in chunks if c[2] == 0]
                        h1 = [c for c in chunks if c[2] == 1]
                        ordered = []
                        a, b = (h0, h1) if len(h0) >= len(h1) else (h1, h0)
                        for i in range(len(chunks)):
                            src = a if i % 2 == 0 else b
                            if not src:
                                src = a if a else b
                            ordered.append(src.pop(0))
                        for off, sg, hh, nh in ordered:
                            score_mm(ps, off, fi, sg, hh, nh)
                        nc.scalar.activation(
                            y[:, c0:c0 + L], ps[:, 0:L], Exp, scale=SCALE,
                        )
                        pull(g_pull)
                        new_done = ((c0 + L) // N) * N
                        for tt in range(done // N, new_done // N):
                            tile_sum(tt)
                            if tt % 2 == 1:
                                finish_slice(tt // 2)
                        done = new_done
            pull(len(pending))

        # f-tile 0's projection ran during the fill (minus Q1); fi+1's
        # projection threads into fi's attn stream an instruction at a time.
        for fi in range(FT):
            interleave = proj_steps(fi + 1) if fi + 1 < FT else []
            if fi == 0:
                interleave = list(f0_q1_steps()) + interleave
            emit_attn(fi, interleave)

    nc.compile()
    return nc


def _prep_inputs(x, W_qkv):
    x = np.asarray(x, dtype=np.float32)
    W = np.asarray(W_qkv, dtype=np.float32)
    # per-fi W column blocks [K_fi (128) | Q_fi (128)], then packed
    # partition-major: w[p, ei, c] = wT[ei*128+p, c]
    wq = W[0:768].reshape(FT, 128, E)        # Q blocks per f-tile
    wk = W[768:1536].reshape(FT, 128, E)     # K blocks per f-tile
    wkq = np.stack([wk, wq], axis=1)         # [fi, kq, 128, e]
    wT = wkq.transpose(3, 0, 1, 2).reshape(E, 2 * 128 * FT)  # [e, cols]
    wP = wT.reshape(ET, 128, 2 * 128 * FT).transpose(1, 0, 2)  # [p, ei, cols]
    wA = np.ascontiguousarray(wP[:, :, 0:256].reshape(128, -1)).astype(np.float16)
    wB1 = np.ascontiguousarray(wP[:, :, 256:512].reshape(128, -1)).astype(np.float16)
    wB2 = np.ascontiguousarray(wP[:, :, 512:1536].reshape(128, -1)).astype(np.float16)
    in_maps = []
    for b in range(B):
        xT = x[b].T                           # [e, n]
        xP = np.ascontiguousarray(
            xT.reshape(ET, 128, N).transpose(1, 0, 2).reshape(128, -1)
        ).astype(np.float16)
        in_maps.append({"xP": xP, "wA": wA, "wB1": wB1, "wB2": wB2})
    return in_maps


def _postprocess(res):
    outs = []
    inv = np.float32(1.0 / OUT_SCALE)
    for r in res.results:
        buf = r["out"]            # [48, 128, 2048] fp16, = out*1024
        buf = np.asarray(buf)
        if buf.dtype != np.float16:
            buf = buf.view(np.float16)
        full = buf.reshape(FT, 8, 128, 2, N).transpose(0, 3, 1, 2, 4)
        full = full.reshape(H, N, N).astype(np.float32) * inv
        outs.append(full)
    return np.stack(outs, axis=0)


def _run(x, W_qkv, trace=False):
    if "nc" not in _cache:
        _cache["nc"] = _build()
    nc = _cache["nc"]
    in_maps = _prep_inputs(x, W_qkv)
    res = run_bass_kernel_spmd(nc, in_maps, core_ids=list(range(B)), trace=trace)
    return _postprocess(res), res


def kernel(x, W_qkv):
    return _run(x, W_qkv)[0]


# revision 35
# speedup vs baseline: 1.1398x; 1.1398x over previous
"""Trainium2 Bass kernel for nn_Attention_layer_67877663146058.

Computes attn = softmax((x @ Wq.T) @ (x @ Wk.T)^T * hd**-0.5)
for x [8, 1024, 768], W_qkv [2304, 768] -> out [8, 12, 1024, 1024] fp32.
The V third of W_qkv never reaches the output and is not loaded.

Sharding: batch-parallel across the 8 NeuronCores (core b handles batch b,
all 12 heads).

Design notes (evolved from trace analysis across many iterations):
- The kernel is ACT(exp)-paced: 96 x [128,1024] exp tiles per core at the
  1x dtype-independent ACT rate (~1.03ns/elem + ~183ns/instr) is ~97us;
  everything else is scheduled to hide under it. DVE (~101us: identity
  row-sums at 1x, proj casts, recips) and PE (~103us incl. exposed
  drains) are near-co-bound; GpSimd carries most normalize multiplies.
- Output is fp16 scaled by 1024 (smallest softmax entries ~2e-6 would be
  fp16 subnormals); the host upcast multiplies by the exact 1/1024.
- Row sums: A-slices (3,6,7) use the free ACT accumulator (standalone
  [128,1024] exps + 182ns READ_ACCUMULATOR); G-runs tile cleanly as
  6144=4x1536 / 4096=2x1536+1024 spans with DVE tensor_scalar+accum sums.
  The last f-tile is A-heavy (3..7): its DVE would otherwise saturate on
  sums and delay the final normalize+DMA drain. Its normalize pairs are
  split GpSimd/DVE so they run concurrently, and the final slice's DMA
  is split per-1024 so the last transfer is small. Drain after the last
  exp is ~6.5us including ~3us of fixed end-of-NEFF overhead.
- Input DMA: one HWDGE ring only sustains ~210GB/s on HBM reads, so the
  fill-critical wt0+x (1.9MB) is split across the Sync AND Scalar rings
  (~350GB/s combined); each ring drains FIFO so the remaining weights
  (wt1 f-tile 1 / wt2 rest, needed ~17us+ in) queue strictly behind x
  and cannot round-robin bandwidth away from it.
- Fill: the f-tile-0 projection runs e-tile-outer over K0/K1/Q0 so its
  matmuls overlap the x chunk arrivals; Q1 (first needed by slice 4) is
  deferred into the interleave stream. The tail x chunk is quarter-major
  so the K0->K1->Q0 casts cascade early (K0 on the otherwise-idle ACT,
  rest on DVE). A dependency-free exp at t=0 preloads the ACT spline
  table; 9 dummy matmuls bridge PE HAM (a >3.4us PE-idle window would
  re-throttle the clock 2.4->1.2GHz and double the fill matmul time).
- fi+1's projection threads into fi's span stream as deprioritized
  pulled steps (tc.high_priority(-25)) so the scheduler prefers the
  imminent span's score matmuls; an in-order-PE-queue stall on a proj
  matmul whose PSUM slot awaits its DVE cast is the main residual ACT
  gap mechanism (~6-8us across f0-f2).
- Score matmuls put the two heads of an f-tile in PE row groups 0-63 /
  64-127 (tile_position) and alternate row groups within a span so
  adjacent matmuls can overlap in the array.
- PSUM: 2x [128,512] proj slots + 2x [128,1536] score slots = 8 banks.
"""

import numpy as np
from contextlib import ExitStack

import concourse.bacc as bacc
import concourse.mybir as mybir
import concourse.tile as tile

# bass_utils imports antenv.axon_hooks when BASS_TRACE is set in the
# environment; some images ship an antenv stub without that module. Register
# a no-op fallback so tracing degrades gracefully instead of crashing.
try:
    from antenv.axon_hooks import get_axon_ntff_profile_hook as _g  # noqa: F401
except Exception:
    import sys as _sys
    import types as _types

    _m = _types.ModuleType("antenv.axon_hooks")
    _state = {"h": None}
    _m.set_axon_ntff_profile_hook = lambda h: _state.__setitem__("h", h)
    _m.get_axon_ntff_profile_hook = lambda: _state["h"]
    _sys.modules["antenv.axon_hooks"] = _m
    try:
        import antenv as _antenv

        _antenv.axon_hooks = _m
    except Exception:
        pass

from concourse.bass_utils import run_bass_kernel_spmd

B = 8          # batches == cores
N = 1024       # tokens
E = 768        # embed dim
H = 12         # heads
HD = 64        # head dim
FT = 6         # f-tiles (2 heads per f-tile)
ET = E // 128  # 6 e-tiles
SPAN = 1536    # G-region exp span (one PSUM score slot, 3 banks)
SCALE = HD ** -0.5
OUT_SCALE = 1024.0   # fp16 output holds out*1024 to avoid subnormals

# qb slices whose sums come from the ACT accumulator. (3,6,7) tiles the
# G region as 6144=4x1536 + 4096=2x1536+1024 (no 512 fragments) and ends
# every f-tile with two self-contained A-slices, so the last f-tile's
# drain after the final exp is just recip+mult+DMA. The last f-tile uses
# more A-slices: its DVE would otherwise saturate on sums and delay the
# final normalize+DMA chain.
A_SLICES = (3, 6, 7)
A_SLICES_LAST = (3, 4, 5, 6, 7)
# pulled projection steps are deprioritized by this many emission slots
# so the scheduler runs the imminent span's score matmuls first
PULL_DEPRIO = 25
# steady-f-tile slices whose normalize mults run on DVE (rest GpSimd);
# the last f-tile splits every slice's pair across both engines instead
DVE_MULT_STEADY = (3,)

_cache = {}


def _build():
    f32 = mybir.dt.float32
    f16 = mybir.dt.float16
    mult = mybir.AluOpType.mult
    add = mybir.AluOpType.add
    Exp = mybir.ActivationFunctionType.Exp
    nc = bacc.Bacc("TRN2", debug=False, num_devices=B)

    # inputs are packed partition-major on the host so each DMA moves
    # multi-KB contiguous runs per partition (128 fat descriptors/DMA)
    xP_d = nc.dram_tensor("xP", [128, ET * N], f16, kind="ExternalInput")
    wA_d = nc.dram_tensor("wA", [128, ET * 256], f16, kind="ExternalInput")
    wB1_d = nc.dram_tensor("wB1", [128, ET * 256], f16, kind="ExternalInput")
    wB2_d = nc.dram_tensor("wB2", [128, ET * 1024], f16, kind="ExternalInput")
    out_d = nc.dram_tensor("out", [FT * 8, 128, 2048], f16, kind="ExternalOutput")

    xP_src = xP_d.ap().rearrange("p (t n) -> p t n", t=ET)      # [128,6,1024]
    wA_src = wA_d.ap().rearrange("p (t c) -> p t c", t=ET)      # [128,6,256]
    wB1_src = wB1_d.ap().rearrange("p (t c) -> p t c", t=ET)    # [128,6,256]
    wB2_src = wB2_d.ap().rearrange("p (t c) -> p t c", t=ET)    # [128,6,1024]
    out_ap = out_d.ap()

    with ExitStack() as ctx:
        tc = ctx.enter_context(tile.TileContext(nc))
        statics = ctx.enter_context(tc.tile_pool(name="statics", bufs=1))
        ypool = ctx.enter_context(tc.tile_pool(name="ypool", bufs=2))
        spool = ctx.enter_context(tc.tile_pool(name="spool", bufs=2))
        psum = ctx.enter_context(tc.tile_pool(name="psum", bufs=2, space="PSUM"))

        xt = statics.tile([128, ET, N], f16, tag="xt", name="xt")
        # W columns split into three contiguous tiles (f-tile 0 / 1 / rest)
        # so each input DMA moves multi-KB runs per partition and f-tile 1's
        # weights (needed first for the interleaved projection) land with
        # their own completion instead of behind the whole remainder
        wt0 = statics.tile([128, ET, 256], f16, tag="wt0", name="wt0")
        wt1 = statics.tile([128, ET, 256], f16, tag="wt1", name="wt1")
        wt2 = statics.tile([128, ET, 1024], f16, tag="wt2", name="wt2")
        # qkt[:, fi, 0, :] = K^T of f-tile fi, qkt[:, fi, 1, :] = Q^T
        qkt = statics.tile([128, FT, 2, N], f16, tag="qkt", name="qkt")

        def w_block(fi, kq, ei):
            if fi == 0:
                return wt0[:, ei, kq * 128:(kq + 1) * 128]
            if fi == 1:
                return wt1[:, ei, kq * 128:(kq + 1) * 128]
            c = (fi - 2) * 256 + kq * 128
            return wt2[:, ei, c:c + 128]

        # ACT table preload: dependency-free exp at t=0 pulls the ~2.7us
        # ACT_TABLE_LOAD off the critical path of the first real exp.
        warm = spool.tile([128, 1], f32, tag="warm", name="warm")
        nc.vector.memset(warm, 0.0)

        # PE HAM warm-up weights
        wl = statics.tile([128, 128], f16, tag="wl", name="wl")
        wr = statics.tile([128, 512], f16, tag="wr", name="wr")
        nc.vector.memset(wl, 0.0)
        nc.vector.memset(wr, 0.0)

        # Input loads. A single HWDGE ring only sustains ~210 GB/s on HBM
        # reads, so the fill-critical wt0+x (1.9MB) is split across BOTH
        # rings (Sync + Scalar, ~350 GB/s combined). Each ring drains its
        # entries FIFO, so the remaining weights (needed only ~17us in)
        # sit strictly BEHIND x on the Scalar ring and cannot round-robin
        # bandwidth away from it.
        nc.sync.dma_start(wt0, wA_src)
        nc.scalar.dma_start(xt[:, 0:2, :], xP_src[:, 0:2, :])
        nc.sync.dma_start(xt[:, 2:4, :], xP_src[:, 2:4, :])
        nc.scalar.dma_start(xt[:, 4:6, :], xP_src[:, 4:6, :])
        nc.scalar.dma_start(wt1, wB1_src)
        nc.scalar.dma_start(wt2, wB2_src)

        nc.scalar.activation(warm, warm, Exp)

        # PE HAM warm-up: dummy matmuls keep the PE busy (and its clock
        # gate at 2.4 GHz) until the first x chunk lands; a >3.4us PE-idle
        # window would re-throttle it to 1.2 GHz and double the fill
        # projection's matmul time. Heavily deprioritized so the scheduler
        # only slots them where the PE would otherwise idle — they must
        # never delay a real matmul. Parks in a "ps" slot.
        pw = psum.tile([128, SPAN], f32, tag="ps", name="pw")
        for _ in range(9):
            nc.tensor.matmul(pw[:, 0:512], lhsT=wl, rhs=wr, start=True,
                             stop=True, skip_group_check=True)

        QUARTERS = [(0, 0), (0, 1), (1, 0), (1, 1)]  # K halves first

        # f-tile 0 projection, e-tile outer: the K0/K1/Q0 quarter
        # accumulations proceed as each x chunk lands instead of waiting
        # for all of x. K quarters borrow the (idle) score slots so their
        # casts free them for the first spans; Q0 sits in a proj slot.
        # Q1 (first needed by slice 4, several spans in) is deferred into
        # f-tile 0's interleave stream entirely.
        FILL_Q = [(0, 0, "ps"), (0, 1, "ps"), (1, 0, "pp")]
        f0_tiles = []
        for qi, (kq, half, tag) in enumerate(FILL_Q):
            shape = [128, SPAN] if tag == "ps" else [128, 512]
            f0_tiles.append(psum.tile(shape, f32, tag=tag, name=f"f0q{qi}"))

        def f0_mm(qi, ei):
            kq, half, _ = FILL_Q[qi]
            nc.tensor.matmul(
                f0_tiles[qi][:, 0:512],
                lhsT=w_block(0, kq, ei),
                rhs=xt[:, ei, half * 512:(half + 1) * 512],
                start=(ei == 0),
                stop=(ei == ET - 1),
                skip_group_check=True,
            )

        for ei in range(4):
            for qi in range(3):
                f0_mm(qi, ei)
        # last x chunk: quarter-major so K0 finishes first and its cast
        # overlaps the remaining matmuls
        for qi in range(3):
            for ei in (4, 5):
                f0_mm(qi, ei)
        # Casts needed before the first span: K both halves + the first
        # 256 Q columns (the first span's lhsT only reads qb0/qb1). Split
        # across the otherwise-idle ACT engine and DVE.
        nc.scalar.copy(qkt[:, 0, 0, 0:512], f0_tiles[0][:, 0:512])
        nc.vector.tensor_copy(qkt[:, 0, 0, 512:1024], f0_tiles[1][:, 0:512])
        nc.vector.tensor_copy(qkt[:, 0, 1, 0:256], f0_tiles[2][:, 0:256])
        nc.vector.tensor_copy(qkt[:, 0, 1, 256:512], f0_tiles[2][:, 256:512])

        fence_t = spool.tile([128, 8], f32, tag="fence", name="fence")

        def q0_fence():
            # tiny extra reader of Q0's proj PSUM: its mid-f0 pull position
            # keeps the pp slot occupied so f1's proj matmuls can't crowd
            # ahead of f-tile 0's first span matmuls in the baked PE order
            nc.vector.tensor_copy(fence_t, f0_tiles[2][:, 0:8])

        def f0_q1_steps():
            holder = []
            for ei in range(ET):
                def mm_step(ei=ei, holder=holder):
                    if ei == 0:
                        holder.append(psum.tile(
                            [128, 512], f32, tag="pp", name="f0q3"))
                    nc.tensor.matmul(
                        holder[0],
                        lhsT=w_block(0, 1, ei),
                        rhs=xt[:, ei, 512:1024],
                        start=(ei == 0),
                        stop=(ei == ET - 1),
                        skip_group_check=True,
                    )
                yield mm_step

            def cast_step(holder=holder):
                nc.vector.tensor_copy(qkt[:, 0, 1, 512:1024], holder[0])
            yield cast_step

        def proj_steps(fi):
            # The projection for f-tile fi as a list of single-instruction
            # closures (6 accumulating matmuls + 1 evacuation cast per
            # quarter). Threading these one or two at a time between score
            # spans keeps the PE from blocking the ACT-paced span stream
            # (accumulation groups don't need to be contiguous in the PE
            # program; they only own their PSUM bank).
            steps = []
            for kq, half in QUARTERS:
                holder = []
                for ei in range(ET):
                    def mm_step(kq=kq, half=half, ei=ei, holder=holder):
                        if ei == 0:
                            holder.append(psum.tile(
                                [128, 512], f32, tag="pp",
                                name=f"pp{fi}_{kq}_{half}",
                            ))
                        nc.tensor.matmul(
                            holder[0],
                            lhsT=w_block(fi, kq, ei),
                            rhs=xt[:, ei, half * 512:(half + 1) * 512],
                            start=(ei == 0),
                            stop=(ei == ET - 1),
                            skip_group_check=True,
                        )
                    steps.append(mm_step)

                def cast_step(kq=kq, half=half, holder=holder):
                    nc.vector.tensor_copy(
                        qkt[:, fi, kq, half * 512:(half + 1) * 512],
                        holder[0],
                    )
                steps.append(cast_step)
            return steps

        def score_mm(ps, off, fi, qb, hh, nh):
            lo = 64 * hh
            nc.tensor.matmul(
                ps[:, off:off + 512],
                lhsT=qkt[lo:lo + 64, fi, 1, qb * 128:(qb + 1) * 128],
                rhs=qkt[lo:lo + 64, fi, 0, nh * 512:(nh + 1) * 512],
                start=True,
                stop=True,
                tile_position=(lo, 0),
            )

        def emit_attn(fi, interleave):
            # Slot s of y holds qb s (identity layout). A-slices (ACT
            # accumulator, self-contained 1024-col exps) are spaced between
            # span-aligned G-runs; the G-region generates more DVE work per
            # ACT span than ACT consumes (identity sums + recip + casts), so
            # A-slices let the DVE queue drain.
            a_last = fi == FT - 1
            a_set = A_SLICES_LAST if a_last else A_SLICES
            segs = []
            run = []
            for s in range(8):
                if s in a_set:
                    if run:
                        segs.append(("G", run))
                        run = []
                    segs.append(("A", [s]))
                else:
                    run.append(s)
            if run:
                segs.append(("G", run))
            if fi == 0:
                # lead with the A3 slice: its exps need only 4 matmuls
                # after the fill casts, so ACT starts earliest
                segs = ([sg for sg in segs if sg == ("A", [3])]
                        + [sg for sg in segs if sg != ("A", [3])])

            y = ypool.tile([128, 16 * N], f16, tag="y", name=f"y{fi}")
            sums = spool.tile([128, 16], f32, tag="sums", name=f"sm{fi}")
            rec = spool.tile([128, 16], f32, tag="rec", name=f"rc{fi}")
            pending = list(interleave)

            def pull(k):
                # deprioritized: the pulled proj matmuls must not crowd
                # ahead of the next span's score matmuls in the PE queue
                with tc.high_priority(-PULL_DEPRIO):
                    for fn in pending[:k]:
                        fn()
                del pending[:k]

            def tile_sum(tt):
                yt = y[:, tt * N:(tt + 1) * N]
                nc.vector.tensor_scalar(
                    yt, yt, 1.0, 0.0, mult, add, accum_out=sums[:, tt:tt + 1],
                )

            def finish_slice(s):
                pull(1)
                t0 = 2 * s
                nc.vector.reciprocal(rec[:, t0:t0 + 2], sums[:, t0:t0 + 2])
                split_dma = a_last and s == 7
                for tt in (t0, t0 + 1):
                    yt = y[:, tt * N:(tt + 1) * N]
                    if a_last:
                        # both engines per slice: the pair runs concurrently,
                        # shortening the drain after the last exp
                        eng = nc.gpsimd if tt == t0 else nc.vector
                    else:
                        eng = nc.vector if s in DVE_MULT_STEADY else nc.gpsimd
                    eng.tensor_scalar(yt, yt, rec[:, tt:tt + 1], OUT_SCALE,
                                      mult, mult)
                    if split_dma:
                        nc.sync.dma_start(
                            out_ap[fi * 8 + s][:, (tt - t0) * N:(tt - t0 + 1) * N],
                            yt,
                        )
                if not split_dma:
                    nc.sync.dma_start(
                        out_ap[fi * 8 + s], y[:, s * 2048:(s + 1) * 2048]
                    )

            # Pulled proj steps land in the PE queue between the emitting
            # span's matmuls and the NEXT span's; inside an A-slice they
            # would delay the following G-span's matmuls past its exp and
            # open an ACT gap, so A-slices don't pull (except f-tile 0,
            # which has more steps to thread).
            a_pull = 2 if fi == 0 else 0
            g_pull = 2 if fi == 0 else 3
            for kind, qbs in segs:
                if kind == "A":
                    s = qbs[0]
                    for tt in (2 * s, 2 * s + 1):
                        hh = tt % 2
                        ps = psum.tile([128, SPAN], f32, tag="ps",
                                       name=f"psA{fi}_{tt}")
                        for nh in range(2):
                            score_mm(ps, nh * 512, fi, s, hh, nh)
                        nc.scalar.activation(
                            y[:, tt * N:(tt + 1) * N], ps[:, 0:N], Exp,
                            scale=SCALE, accum_out=sums[:, tt:tt + 1],
                        )
                        pull(a_pull)
                    finish_slice(s)
                else:
                    run0 = qbs[0] * 2048
                    run1 = run0 + len(qbs) * 2048
                    done = run0
                    for c0 in range(run0, run1, SPAN):
                        L = min(SPAN, run1 - c0)
                        ps = psum.tile([128, SPAN], f32, tag="ps",
                                       name=f"psG{fi}_{c0}")
                        # order chunks so PE row groups (hh) alternate:
                        # adjacent matmuls then run concurrently in the
                        # 128x128 array.
                        chunks = []
                        for off in range(0, L, 512):
                            g = c0 + off
                            chunks.append(
                                (off, g // 2048, (g // 1024) % 2, (g // 512) % 2))
                        h0 = [c for c in chunks if c[2] == 0]
                        h1 = [c for c in chunks if c[2] == 1]
                        ordered = []
                        a, b = (h0, h1) if len(h0) >= len(h1) else (h1, h0)
                        for i in range(len(chunks)):
                            src = a if i % 2 == 0 else b
                            if not src:
                                src = a if a else b
                            ordered.append(src.pop(0))
                        for off, sg, hh, nh in ordered:
                            score_mm(ps, off, fi, sg, hh, nh)
                        nc.scalar.activation(
                            y[:, c0:c0 + L], ps[:, 0:L], Exp, scale=SCALE,
                        )
                        pull(g_pull)
                        new_done = ((c0 + L) // N) * N
                        for tt in range(done // N, new_done // N):
                            tile_sum(tt)
                            if tt % 2 == 1:
                                finish_slice(tt // 2)
                        done = new_done
            pull(len(pending))

        # f-tile 0's projection ran during the fill (minus Q1); fi+1's
        # projection threads into fi's attn stream an instruction at a time.
        for fi in range(FT):
            interleave = proj_steps(fi + 1) if fi + 1 < FT else []
            if fi == 0:
                interleave = list(f0_q1_steps()) + [q0_fence] + interleave
            emit_attn(fi, interleave)

    nc.compile()
    return nc


def _prep_inputs(x, W_qkv):
    x = np.asarray(x, dtype=np.float32)
    W = np.asarray(W_qkv, dtype=np.float32)
    # per-fi W column blocks [K_fi (128) | Q_fi (128)], then packed
    # partition-major: w[p, ei, c] = wT[ei*128+p, c]
    wq = W[0:768].reshape(FT, 128, E)        # Q blocks per f-tile
    wk = W[768:1536].reshape(FT, 128, E)     # K blocks per f-tile
    wkq = np.stack([wk, wq], axis=1)         # [fi, kq, 128, e]
    wT = wkq.transpose(3, 0, 1, 2).reshape(E, 2 * 128 * FT)  # [e, cols]
    wP = wT.reshape(ET, 128, 2 * 128 * FT).transpose(1, 0, 2)  # [p, ei, cols]
    wA = np.ascontiguousarray(wP[:, :, 0:256].reshape(128, -1)).astype(np.float16)
    wB1 = np.ascontiguousarray(wP[:, :, 256:512].reshape(128, -1)).astype(np.float16)
    wB2 = np.ascontiguousarray(wP[:, :, 512:1536].reshape(128, -1)).astype(np.float16)
    in_maps = []
    for b in range(B):
        xT = x[b].T                           # [e, n]
        xP = np.ascontiguousarray(
            xT.reshape(ET, 128, N).transpose(1, 0, 2).reshape(128, -1)
        ).astype(np.float16)
        in_maps.append({"xP": xP, "wA": wA, "wB1": wB1, "wB2": wB2})
    return in_maps


def _postprocess(res):
    outs = []
    inv = np.float32(1.0 / OUT_SCALE)
    for r in res.results:
        buf = r["out"]            # [48, 128, 2048] fp16, = out*1024
        buf = np.asarray(buf)
        if buf.dtype != np.float16:
            buf = buf.view(np.float16)
        full = buf.reshape(FT, 8, 128, 2, N).transpose(0, 3, 1, 2, 4)
        full = full.reshape(H, N, N).astype(np.float32) * inv
        outs.append(full)
    return np.stack(outs, axis=0)


def _run(x, W_qkv, trace=False):
    if "nc" not in _cache:
        _cache["nc"] = _build()
    nc = _cache["nc"]
    in_maps = _prep_inputs(x, W_qkv)
    res = run_bass_kernel_spmd(nc, in_maps, core_ids=list(range(B)), trace=trace)
    return _postprocess(res), res


def kernel(x, W_qkv):
    return _run(x, W_qkv)[0]


# revision 36
# speedup vs baseline: 1.1717x; 1.0280x over previous
"""Trainium2 Bass kernel for nn_Attention_layer_67877663146058.

Computes attn = softmax((x @ Wq.T) @ (x @ Wk.T)^T * hd**-0.5)
for x [8, 1024, 768], W_qkv [2304, 768] -> out [8, 12, 1024, 1024] fp32.
The V third of W_qkv never reaches the output and is not loaded.

Sharding: batch-parallel across the 8 NeuronCores (core b handles batch b,
all 12 heads).

Design notes (evolved from trace analysis across many iterations):
- The kernel is ACT(exp)-paced: 96 x [128,1024] exp tiles per core at the
  1x dtype-independent ACT rate (~1.03ns/elem + ~183ns/instr) is ~97us;
  everything else is scheduled to hide under it. DVE (~101us: identity
  row-sums at 1x, proj casts, recips) and PE (~103us incl. exposed
  drains) are near-co-bound; GpSimd carries most normalize multiplies.
- Output is fp16 scaled by 1024 (smallest softmax entries ~2e-6 would be
  fp16 subnormals); the host upcast multiplies by the exact 1/1024.
- Row sums: A-slices (3,6,7) use the free ACT accumulator (standalone
  [128,1024] exps + 182ns READ_ACCUMULATOR); G-runs tile cleanly as
  6144=4x1536 / 4096=2x1536+1024 spans with DVE tensor_scalar+accum sums.
  The last f-tile is A-heavy (3..7): its DVE would otherwise saturate on
  sums and delay the final normalize+DMA drain. Its normalize pairs are
  split GpSimd/DVE so they run concurrently, and the final slice's DMA
  is split per-1024 so the last transfer is small. Drain after the last
  exp is ~6.5us including ~3us of fixed end-of-NEFF overhead.
- Input DMA: one HWDGE ring only sustains ~210GB/s on HBM reads, so the
  fill-critical wt0+x (1.9MB) is split across the Sync AND Scalar rings
  (~350GB/s combined); each ring drains FIFO so the remaining weights
  (wt1 f-tile 1 / wt2 rest, needed ~17us+ in) queue strictly behind x
  and cannot round-robin bandwidth away from it.
- Fill: the f-tile-0 projection runs e-tile-outer over K0/K1/Q0 so its
  matmuls overlap the x chunk arrivals; Q1 (first needed by slice 4) is
  deferred into the interleave stream. The tail x chunk is quarter-major
  so the K0->K1->Q0 casts cascade early (K0 on the otherwise-idle ACT,
  rest on DVE). A dependency-free exp at t=0 preloads the ACT spline
  table; 9 dummy matmuls bridge PE HAM (a >3.4us PE-idle window would
  re-throttle the clock 2.4->1.2GHz and double the fill matmul time).
- fi+1's projection threads into fi's span stream as deprioritized
  pulled steps (tc.high_priority(-25)) so the scheduler prefers the
  imminent span's score matmuls; an in-order-PE-queue stall on a proj
  matmul whose PSUM slot awaits its DVE cast is the main residual ACT
  gap mechanism (~6-8us across f0-f2).
- Score matmuls put the two heads of an f-tile in PE row groups 0-63 /
  64-127 (tile_position) and alternate row groups within a span so
  adjacent matmuls can overlap in the array.
- PSUM: 2x [128,512] proj slots + 2x [128,1536] score slots = 8 banks.
"""

import numpy as np
from contextlib import ExitStack

import concourse.bacc as bacc
import concourse.mybir as mybir
import concourse.tile as tile

# bass_utils imports antenv.axon_hooks when BASS_TRACE is set in the
# environment; some images ship an antenv stub without that module. Register
# a no-op fallback so tracing degrades gracefully instead of crashing.
try:
    from antenv.axon_hooks import get_axon_ntff_profile_hook as _g  # noqa: F401
except Exception:
    import sys as _sys
    import types as _types

    _m = _types.ModuleType("antenv.axon_hooks")
    _state = {"h": None}
    _m.set_axon_ntff_profile_hook = lambda h: _state.__setitem__("h", h)
    _m.get_axon_ntff_profile_hook = lambda: _state["h"]
    _sys.modules["antenv.axon_hooks"] = _m
    try:
        import antenv as _antenv

        _antenv.axon_hooks = _m
    except Exception:
        pass

from concourse.bass_utils import run_bass_kernel_spmd

B = 8          # batches == cores
N = 1024       # tokens
E = 768        # embed dim
H = 12         # heads
HD = 64        # head dim
FT = 6         # f-tiles (2 heads per f-tile)
ET = E // 128  # 6 e-tiles
SPAN = 1536    # G-region exp span (one PSUM score slot, 3 banks)
SCALE = HD ** -0.5
OUT_SCALE = 1024.0   # fp16 output holds out*1024 to avoid subnormals

# qb slices whose sums come from the ACT accumulator. (3,6,7) tiles the
# G region as 6144=4x1536 + 4096=2x1536+1024 (no 512 fragments) and ends
# every f-tile with two self-contained A-slices, so the last f-tile's
# drain after the final exp is just recip+mult+DMA. The last f-tile uses
# more A-slices: its DVE would otherwise saturate on sums and delay the
# final normalize+DMA chain.
A_SLICES = (3, 6, 7)
A_SLICES_LAST = (3, 4, 5, 6, 7)
# pulled projection steps are deprioritized by this many emission slots
# so the scheduler runs the imminent span's score matmuls first
PULL_DEPRIO = 25
# steady-f-tile slices whose normalize mults run on DVE (rest GpSimd);
# the last f-tile splits every slice's pair across both engines instead
DVE_MULT_STEADY = (3,)

_cache = {}


def _build():
    f32 = mybir.dt.float32
    f16 = mybir.dt.float16
    mult = mybir.AluOpType.mult
    add = mybir.AluOpType.add
    Exp = mybir.ActivationFunctionType.Exp
    nc = bacc.Bacc("TRN2", debug=False, num_devices=B)

    # inputs are packed partition-major on the host so each DMA moves
    # multi-KB contiguous runs per partition (128 fat descriptors/DMA)
    xP_d = nc.dram_tensor("xP", [128, ET * N], f16, kind="ExternalInput")
    wA_d = nc.dram_tensor("wA", [128, ET * 256], f16, kind="ExternalInput")
    wB1_d = nc.dram_tensor("wB1", [128, ET * 256], f16, kind="ExternalInput")
    wB2_d = nc.dram_tensor("wB2", [128, ET * 1024], f16, kind="ExternalInput")
    out_d = nc.dram_tensor("out", [FT * 8, 128, 2048], f16, kind="ExternalOutput")

    xP_src = xP_d.ap().rearrange("p (t n) -> p t n", t=ET)      # [128,6,1024]
    wA_src = wA_d.ap().rearrange("p (t c) -> p t c", t=ET)      # [128,6,256]
    wB1_src = wB1_d.ap().rearrange("p (t c) -> p t c", t=ET)    # [128,6,256]
    wB2_src = wB2_d.ap().rearrange("p (t c) -> p t c", t=ET)    # [128,6,1024]
    out_ap = out_d.ap()

    with ExitStack() as ctx:
        tc = ctx.enter_context(tile.TileContext(nc))
        statics = ctx.enter_context(tc.tile_pool(name="statics", bufs=1))
        ypool = ctx.enter_context(tc.tile_pool(name="ypool", bufs=2))
        spool = ctx.enter_context(tc.tile_pool(name="spool", bufs=2))
        psum = ctx.enter_context(tc.tile_pool(name="psum", bufs=2, space="PSUM"))

        xt = statics.tile([128, ET, N], f16, tag="xt", name="xt")
        # W columns split into three contiguous tiles (f-tile 0 / 1 / rest)
        # so each input DMA moves multi-KB runs per partition and f-tile 1's
        # weights (needed first for the interleaved projection) land with
        # their own completion instead of behind the whole remainder
        wt0 = statics.tile([128, ET, 256], f16, tag="wt0", name="wt0")
        wt1 = statics.tile([128, ET, 256], f16, tag="wt1", name="wt1")
        wt2 = statics.tile([128, ET, 1024], f16, tag="wt2", name="wt2")
        # qkt[:, fi, 0, :] = K^T of f-tile fi, qkt[:, fi, 1, :] = Q^T
        qkt = statics.tile([128, FT, 2, N], f16, tag="qkt", name="qkt")

        def w_block(fi, kq, ei):
            if fi == 0:
                return wt0[:, ei, kq * 128:(kq + 1) * 128]
            if fi == 1:
                return wt1[:, ei, kq * 128:(kq + 1) * 128]
            c = (fi - 2) * 256 + kq * 128
            return wt2[:, ei, c:c + 128]

        # ACT table preload: dependency-free exp at t=0 pulls the ~2.7us
        # ACT_TABLE_LOAD off the critical path of the first real exp.
        warm = spool.tile([128, 1], f32, tag="warm", name="warm")
        nc.vector.memset(warm, 0.0)

        # PE HAM warm-up weights
        wl = statics.tile([128, 128], f16, tag="wl", name="wl")
        wr = statics.tile([128, 512], f16, tag="wr", name="wr")
        nc.vector.memset(wl, 0.0)
        nc.vector.memset(wr, 0.0)

        # Input loads. A single HWDGE ring only sustains ~210 GB/s on HBM
        # reads, so the fill-critical wt0+x (1.9MB) is split across BOTH
        # rings (Sync + Scalar, ~350 GB/s combined). Each ring drains its
        # entries FIFO, so the remaining weights (needed only ~17us in)
        # sit strictly BEHIND x on the Scalar ring and cannot round-robin
        # bandwidth away from it.
        nc.sync.dma_start(wt0, wA_src)
        nc.scalar.dma_start(xt[:, 0:2, :], xP_src[:, 0:2, :])
        nc.sync.dma_start(xt[:, 2:4, :], xP_src[:, 2:4, :])
        nc.scalar.dma_start(xt[:, 4:6, :], xP_src[:, 4:6, :])
        nc.scalar.dma_start(wt1, wB1_src)
        nc.scalar.dma_start(wt2, wB2_src)

        nc.scalar.activation(warm, warm, Exp)

        # PE HAM warm-up: dummy matmuls keep the PE busy (and its clock
        # gate at 2.4 GHz) until the first x chunk lands; a >3.4us PE-idle
        # window would re-throttle it to 1.2 GHz and double the fill
        # projection's matmul time. Heavily deprioritized so the scheduler
        # only slots them where the PE would otherwise idle — they must
        # never delay a real matmul. Parks in a "ps" slot.
        pw = psum.tile([128, SPAN], f32, tag="ps", name="pw")
        for _ in range(9):
            nc.tensor.matmul(pw[:, 0:512], lhsT=wl, rhs=wr, start=True,
                             stop=True, skip_group_check=True)

        QUARTERS = [(0, 0), (0, 1), (1, 0), (1, 1)]  # K halves first

        # f-tile 0 projection, e-tile outer: the K0/K1/Q0 quarter
        # accumulations proceed as each x chunk lands instead of waiting
        # for all of x. K quarters borrow the (idle) score slots so their
        # casts free them for the first spans; Q0 sits in a proj slot.
        # Q1 (first needed by slice 4, several spans in) is deferred into
        # f-tile 0's interleave stream entirely.
        FILL_Q = [(0, 0, "ps"), (0, 1, "ps"), (1, 0, "pp")]
        f0_tiles = []
        for qi, (kq, half, tag) in enumerate(FILL_Q):
            shape = [128, SPAN] if tag == "ps" else [128, 512]
            f0_tiles.append(psum.tile(shape, f32, tag=tag, name=f"f0q{qi}"))

        def f0_mm(qi, ei):
            kq, half, _ = FILL_Q[qi]
            nc.tensor.matmul(
                f0_tiles[qi][:, 0:512],
                lhsT=w_block(0, kq, ei),
                rhs=xt[:, ei, half * 512:(half + 1) * 512],
                start=(ei == 0),
                stop=(ei == ET - 1),
                skip_group_check=True,
            )

        for ei in range(4):
            for qi in range(3):
                f0_mm(qi, ei)
        # last x chunk: quarter-major so K0 finishes first and its cast
        # overlaps the remaining matmuls
        for qi in range(3):
            for ei in (4, 5):
                f0_mm(qi, ei)
        # Casts needed before the first span: K both halves + the first
        # 256 Q columns (the first span's lhsT only reads qb0/qb1). Split
        # across the otherwise-idle ACT engine and DVE.
        nc.scalar.copy(qkt[:, 0, 0, 0:512], f0_tiles[0][:, 0:512])
        nc.vector.tensor_copy(qkt[:, 0, 0, 512:1024], f0_tiles[1][:, 0:512])
        nc.vector.tensor_copy(qkt[:, 0, 1, 0:256], f0_tiles[2][:, 0:256])
        nc.vector.tensor_copy(qkt[:, 0, 1, 256:512], f0_tiles[2][:, 256:512])

        def f0_q1_steps():
            holder = []
            for ei in range(ET):
                def mm_step(ei=ei, holder=holder):
                    if ei == 0:
                        holder.append(psum.tile(
                            [128, 512], f32, tag="pp", name="f0q3"))
                    nc.tensor.matmul(
                        holder[0],
                        lhsT=w_block(0, 1, ei),
                        rhs=xt[:, ei, 512:1024],
                        start=(ei == 0),
                        stop=(ei == ET - 1),
                        skip_group_check=True,
                    )
                yield mm_step

            def cast_step(holder=holder):
                nc.vector.tensor_copy(qkt[:, 0, 1, 512:1024], holder[0])
            yield cast_step

        def proj_steps(fi):
            # The projection for f-tile fi as a list of single-instruction
            # closures (6 accumulating matmuls + 1 evacuation cast per
            # quarter). Threading these one or two at a time between score
            # spans keeps the PE from blocking the ACT-paced span stream
            # (accumulation groups don't need to be contiguous in the PE
            # program; they only own their PSUM bank).
            steps = []
            for kq, half in QUARTERS:
                holder = []
                for ei in range(ET):
                    def mm_step(kq=kq, half=half, ei=ei, holder=holder):
                        if ei == 0:
                            holder.append(psum.tile(
                                [128, 512], f32, tag="pp",
                                name=f"pp{fi}_{kq}_{half}",
                            ))
                        nc.tensor.matmul(
                            holder[0],
                            lhsT=w_block(fi, kq, ei),
                            rhs=xt[:, ei, half * 512:(half + 1) * 512],
                            start=(ei == 0),
                            stop=(ei == ET - 1),
                            skip_group_check=True,
                        )
                    steps.append(mm_step)

                def cast_step(kq=kq, half=half, holder=holder):
                    nc.vector.tensor_copy(
                        qkt[:, fi, kq, half * 512:(half + 1) * 512],
                        holder[0],
                    )
                steps.append(cast_step)
            return steps

        def score_mm(ps, off, fi, qb, hh, nh):
            lo = 64 * hh
            nc.tensor.matmul(
                ps[:, off:off + 512],
                lhsT=qkt[lo:lo + 64, fi, 1, qb * 128:(qb + 1) * 128],
                rhs=qkt[lo:lo + 64, fi, 0, nh * 512:(nh + 1) * 512],
                start=True,
                stop=True,
                tile_position=(lo, 0),
            )

        def emit_attn(fi, interleave):
            # Slot s of y holds qb s (identity layout). A-slices (ACT
            # accumulator, self-contained 1024-col exps) are spaced between
            # span-aligned G-runs; the G-region generates more DVE work per
            # ACT span than ACT consumes (identity sums + recip + casts), so
            # A-slices let the DVE queue drain.
            a_last = fi == FT - 1
            a_set = A_SLICES_LAST if a_last else A_SLICES
            segs = []
            run = []
            for s in range(8):
                if s in a_set:
                    if run:
                        segs.append(("G", run))
                        run = []
                    segs.append(("A", [s]))
                else:
                    run.append(s)
            if run:
                segs.append(("G", run))

            y = ypool.tile([128, 16 * N], f16, tag="y", name=f"y{fi}")
            sums = spool.tile([128, 16], f32, tag="sums", name=f"sm{fi}")
            rec = spool.tile([128, 16], f32, tag="rec", name=f"rc{fi}")
            pending = list(interleave)

            def pull(k):
                # deprioritized: the pulled proj matmuls must not crowd
                # ahead of the next span's score matmuls in the PE queue
                with tc.high_priority(-PULL_DEPRIO):
                    for fn in pending[:k]:
                        fn()
                del pending[:k]

            def tile_sum(tt):
                yt = y[:, tt * N:(tt + 1) * N]
                nc.vector.tensor_scalar(
                    yt, yt, 1.0, 0.0, mult, add, accum_out=sums[:, tt:tt + 1],
                )

            def finish_slice(s):
                pull(1)
                t0 = 2 * s
                nc.vector.reciprocal(rec[:, t0:t0 + 2], sums[:, t0:t0 + 2])
                split_dma = a_last and s == 7
                for tt in (t0, t0 + 1):
                    yt = y[:, tt * N:(tt + 1) * N]
                    if a_last:
                        # both engines per slice: the pair runs concurrently,
                        # shortening the drain after the last exp
                        eng = nc.gpsimd if tt == t0 else nc.vector
                    else:
                        eng = nc.vector if s in DVE_MULT_STEADY else nc.gpsimd
                    eng.tensor_scalar(yt, yt, rec[:, tt:tt + 1], OUT_SCALE,
                                      mult, mult)
                    if split_dma:
                        nc.sync.dma_start(
                            out_ap[fi * 8 + s][:, (tt - t0) * N:(tt - t0 + 1) * N],
                            yt,
                        )
                if not split_dma:
                    nc.sync.dma_start(
                        out_ap[fi * 8 + s], y[:, s * 2048:(s + 1) * 2048]
                    )

            # Pulled proj steps land in the PE queue between the emitting
            # span's matmuls and the NEXT span's; inside an A-slice they
            # would delay the following G-span's matmuls past its exp and
            # open an ACT gap, so A-slices don't pull (except f-tile 0,
            # which has more steps to thread).
            a_pull = 2 if fi == 0 else 0
            g_pull = 2 if fi == 0 else 3
            for kind, qbs in segs:
                if kind == "A":
                    s = qbs[0]
                    for tt in (2 * s, 2 * s + 1):
                        hh = tt % 2
                        ps = psum.tile([128, SPAN], f32, tag="ps",
                                       name=f"psA{fi}_{tt}")
                        for nh in range(2):
                            score_mm(ps, nh * 512, fi, s, hh, nh)
                        nc.scalar.activation(
                            y[:, tt * N:(tt + 1) * N], ps[:, 0:N], Exp,
                            scale=SCALE, accum_out=sums[:, tt:tt + 1],
                        )
                        pull(a_pull)
                    finish_slice(s)
                else:
                    run0 = qbs[0] * 2048
                    run1 = run0 + len(qbs) * 2048
                    done = run0
                    for c0 in range(run0, run1, SPAN):
                        L = min(SPAN, run1 - c0)
                        ps = psum.tile([128, SPAN], f32, tag="ps",
                                       name=f"psG{fi}_{c0}")
                        # order chunks so PE row groups (hh) alternate:
                        # adjacent matmuls then run concurrently in the
                        # 128x128 array.
                        chunks = []
                        for off in range(0, L, 512):
                            g = c0 + off
                            chunks.append(
                                (off, g // 2048, (g // 1024) % 2, (g // 512) % 2))
                        h0 = [c for c in chunks if c[2] == 0]
                        h1 = [c for c in chunks if c[2] == 1]
                        ordered = []
                        a, b = (h0, h1) if len(h0) >= len(h1) else (h1, h0)
                        for i in range(len(chunks)):
                            src = a if i % 2 == 0 else b
                            if not src:
                                src = a if a else b
                            ordered.append(src.pop(0))
                        for off, sg, hh, nh in ordered:
                            score_mm(ps, off, fi, sg, hh, nh)
                        nc.scalar.activation(
                            y[:, c0:c0 + L], ps[:, 0:L], Exp, scale=SCALE,
                        )
                        pull(g_pull)
                        new_done = ((c0 + L) // N) * N
                        for tt in range(done // N, new_done // N):
                            tile_sum(tt)
                            if tt % 2 == 1:
                                finish_slice(tt // 2)
                        done = new_done
            pull(len(pending))

        # f-tile 0's projection ran during the fill (minus Q1); fi+1's
        # projection threads into fi's attn stream an instruction at a time.
        for fi in range(FT):
            interleave = proj_steps(fi + 1) if fi + 1 < FT else []
            if fi == 0:
                interleave = list(f0_q1_steps()) + interleave
            emit_attn(fi, interleave)

    nc.compile()
    return nc


def _prep_inputs(x, W_qkv):
    x = np.asarray(x, dtype=np.float32)
    W = np.asarray(W_qkv, dtype=np.float32)
    # per-fi W column blocks [K_fi (128) | Q_fi (128)], then packed
    # partition-major: w[p, ei, c] = wT[ei*128+p, c]
    wq = W[0:768].reshape(FT, 128, E)        # Q blocks per f-tile
    wk = W[768:1536].reshape(FT, 128, E)     # K blocks per f-tile
    wkq = np.stack([wk, wq], axis=1)         # [fi, kq, 128, e]
    wT = wkq.transpose(3, 0, 1, 2).reshape(E, 2 * 128 * FT)  # [e, cols]
    wP = wT.reshape(ET, 128, 2 * 128 * FT).transpose(1, 0, 2)  # [p, ei, cols]
    wA = np.ascontiguousarray(wP[:, :, 0:256].reshape(128, -1)).astype(np.float16)
    wB1 = np.ascontiguousarray(wP[:, :, 256:512].reshape(128, -1)).astype(np.float16)
    wB2 = np.ascontiguousarray(wP[:, :, 512:1536].reshape(128, -1)).astype(np.float16)
    in_maps = []
    for b in range(B):
        xT = x[b].T                           # [e, n]
        xP = np.ascontiguousarray(
            xT.reshape(ET, 128, N).transpose(1, 0, 2).reshape(128, -1)
        ).astype(np.float16)
        in_maps.append({"xP": xP, "wA": wA, "wB1": wB1, "wB2": wB2})
    return in_maps


def _postprocess(res):
    outs = []
    inv = np.float32(1.0 / OUT_SCALE)
    for r in res.results:
        buf = r["out"]            # [48, 128, 2048] fp16, = out*1024
        buf = np.asarray(buf)
        if buf.dtype != np.float16:
            buf = buf.view(np.float16)
        full = buf.reshape(FT, 8, 128, 2, N).transpose(0, 3, 1, 2, 4)
        full = full.reshape(H, N, N).astype(np.float32) * inv
        outs.append(full)
    return np.stack(outs, axis=0)


def _run(x, W_qkv, trace=False):
    if "nc" not in _cache:
        _cache["nc"] = _build()
    nc = _cache["nc"]
    in_maps = _prep_inputs(x, W_qkv)
    res = run_bass_kernel_spmd(nc, in_maps, core_ids=list(range(B)), trace=trace)
    return _postprocess(res), res


def kernel(x, W_qkv):
    return _run(x, W_qkv)[0]


# revision 37
# speedup vs baseline: 1.2047x; 1.0282x over previous
"""Trainium2 Bass kernel for nn_Attention_layer_67877663146058.

Computes attn = softmax((x @ Wq.T) @ (x @ Wk.T)^T * hd**-0.5)
for x [8, 1024, 768], W_qkv [2304, 768] -> out [8, 12, 1024, 1024] fp32.
The V third of W_qkv never reaches the output and is not loaded.

Sharding: batch-parallel across the 8 NeuronCores (core b handles batch b,
all 12 heads).

Design notes (evolved from trace analysis across many iterations):
- The kernel is ACT(exp)-paced: 96 x [128,1024] exp tiles per core at the
  1x dtype-independent ACT rate (~1.03ns/elem + ~183ns/instr) is ~97us;
  everything else is scheduled to hide under it. DVE (~101us: identity
  row-sums at 1x, proj casts, recips) and PE (~103us incl. exposed
  drains) are near-co-bound; GpSimd carries most normalize multiplies.
- Output is fp16 scaled by 1024 (smallest softmax entries ~2e-6 would be
  fp16 subnormals); the host upcast multiplies by the exact 1/1024.
- Row sums: A-slices (3,6,7) use the free ACT accumulator (standalone
  [128,1024] exps + 182ns READ_ACCUMULATOR); G-runs tile cleanly as
  6144=4x1536 / 4096=2x1536+1024 spans with DVE tensor_scalar+accum sums.
  The last f-tile is A-heavy (3..7): its DVE would otherwise saturate on
  sums and delay the final normalize+DMA drain. Its normalize pairs are
  split GpSimd/DVE so they run concurrently, and the final slice's DMA
  is split per-1024 so the last transfer is small. Drain after the last
  exp is ~6.5us including ~3us of fixed end-of-NEFF overhead.
- Input DMA: one HWDGE ring only sustains ~210GB/s on HBM reads, so the
  fill-critical wt0+x (1.9MB) is split across the Sync AND Scalar rings
  (~350GB/s combined); each ring drains FIFO so the remaining weights
  (wt1 f-tile 1 / wt2 rest, needed ~17us+ in) queue strictly behind x
  and cannot round-robin bandwidth away from it.
- Fill: the f-tile-0 projection runs e-tile-outer over K0/K1/Q0 so its
  matmuls overlap the x chunk arrivals; Q1 (first needed by slice 4) is
  deferred into the interleave stream. The tail x chunk is quarter-major
  so the K0->K1->Q0 casts cascade early (K0 on the otherwise-idle ACT,
  rest on DVE). A dependency-free exp at t=0 preloads the ACT spline
  table; 9 dummy matmuls bridge PE HAM (a >3.4us PE-idle window would
  re-throttle the clock 2.4->1.2GHz and double the fill matmul time).
- fi+1's projection threads into fi's span stream as deprioritized
  pulled steps (tc.high_priority(-25)) so the scheduler prefers the
  imminent span's score matmuls; an in-order-PE-queue stall on a proj
  matmul whose PSUM slot awaits its DVE cast is the main residual ACT
  gap mechanism (~6-8us across f0-f2).
- Score matmuls put the two heads of an f-tile in PE row groups 0-63 /
  64-127 (tile_position) and alternate row groups within a span so
  adjacent matmuls can overlap in the array.
- PSUM: 2x [128,512] proj slots + 2x [128,1536] score slots = 8 banks.
"""

import numpy as np
from contextlib import ExitStack

import concourse.bacc as bacc
import concourse.mybir as mybir
import concourse.tile as tile

# bass_utils imports antenv.axon_hooks when BASS_TRACE is set in the
# environment; some images ship an antenv stub without that module. Register
# a no-op fallback so tracing degrades gracefully instead of crashing.
try:
    from antenv.axon_hooks import get_axon_ntff_profile_hook as _g  # noqa: F401
except Exception:
    import sys as _sys
    import types as _types

    _m = _types.ModuleType("antenv.axon_hooks")
    _state = {"h": None}
    _m.set_axon_ntff_profile_hook = lambda h: _state.__setitem__("h", h)
    _m.get_axon_ntff_profile_hook = lambda: _state["h"]
    _sys.modules["antenv.axon_hooks"] = _m
    try:
        import antenv as _antenv

        _antenv.axon_hooks = _m
    except Exception:
        pass

from concourse.bass_utils import run_bass_kernel_spmd

B = 8          # batches == cores
N = 1024       # tokens
E = 768        # embed dim
H = 12         # heads
HD = 64        # head dim
FT = 6         # f-tiles (2 heads per f-tile)
ET = E // 128  # 6 e-tiles
SPAN = 1536    # G-region exp span (one PSUM score slot, 3 banks)
SCALE = HD ** -0.5
OUT_SCALE = 1024.0   # fp16 output holds out*1024 to avoid subnormals

# qb slices whose sums come from the ACT accumulator. (3,6,7) tiles the
# G region as 6144=4x1536 + 4096=2x1536+1024 (no 512 fragments) and ends
# every f-tile with two self-contained A-slices, so the last f-tile's
# drain after the final exp is just recip+mult+DMA. The last f-tile uses
# more A-slices: its DVE would otherwise saturate on sums and delay the
# final normalize+DMA chain.
A_SLICES = (3, 6, 7)
A_SLICES_LAST = (3, 4, 5, 6, 7)
# pulled projection steps are deprioritized by this many emission slots
# so the scheduler runs the imminent span's score matmuls first
PULL_DEPRIO = 25
# steady-f-tile slices whose normalize mults run on DVE (rest GpSimd);
# the last f-tile splits every slice's pair across both engines instead
DVE_MULT_STEADY = (3,)

_cache = {}


def _build():
    f32 = mybir.dt.float32
    f16 = mybir.dt.float16
    mult = mybir.AluOpType.mult
    add = mybir.AluOpType.add
    Exp = mybir.ActivationFunctionType.Exp
    nc = bacc.Bacc("TRN2", debug=False, num_devices=B)

    # inputs are packed partition-major on the host so each DMA moves
    # multi-KB contiguous runs per partition (128 fat descriptors/DMA)
    xP_d = nc.dram_tensor("xP", [128, ET * N], f16, kind="ExternalInput")
    wA_d = nc.dram_tensor("wA", [128, ET * 256], f16, kind="ExternalInput")
    wB1_d = nc.dram_tensor("wB1", [128, ET * 256], f16, kind="ExternalInput")
    wB2_d = nc.dram_tensor("wB2", [128, ET * 1024], f16, kind="ExternalInput")
    out_d = nc.dram_tensor("out", [FT * 8, 128, 2048], f16, kind="ExternalOutput")

    xP_src = xP_d.ap().rearrange("p (t n) -> p t n", t=ET)      # [128,6,1024]
    wA_src = wA_d.ap().rearrange("p (t c) -> p t c", t=ET)      # [128,6,256]
    wB1_src = wB1_d.ap().rearrange("p (t c) -> p t c", t=ET)    # [128,6,256]
    wB2_src = wB2_d.ap().rearrange("p (t c) -> p t c", t=ET)    # [128,6,1024]
    out_ap = out_d.ap()

    with ExitStack() as ctx:
        tc = ctx.enter_context(tile.TileContext(nc))
        statics = ctx.enter_context(tc.tile_pool(name="statics", bufs=1))
        ypool = ctx.enter_context(tc.tile_pool(name="ypool", bufs=2))
        spool = ctx.enter_context(tc.tile_pool(name="spool", bufs=2))
        psum = ctx.enter_context(tc.tile_pool(name="psum", bufs=2, space="PSUM"))

        xt = statics.tile([128, ET, N], f16, tag="xt", name="xt")
        # W columns split into three contiguous tiles (f-tile 0 / 1 / rest)
        # so each input DMA moves multi-KB runs per partition and f-tile 1's
        # weights (needed first for the interleaved projection) land with
        # their own completion instead of behind the whole remainder
        wt0 = statics.tile([128, ET, 256], f16, tag="wt0", name="wt0")
        wt1 = statics.tile([128, ET, 256], f16, tag="wt1", name="wt1")
        wt2 = statics.tile([128, ET, 1024], f16, tag="wt2", name="wt2")
        # qkt[:, fi, 0, :] = K^T of f-tile fi, qkt[:, fi, 1, :] = Q^T
        qkt = statics.tile([128, FT, 2, N], f16, tag="qkt", name="qkt")

        def w_block(fi, kq, ei):
            if fi == 0:
                return wt0[:, ei, kq * 128:(kq + 1) * 128]
            if fi == 1:
                return wt1[:, ei, kq * 128:(kq + 1) * 128]
            c = (fi - 2) * 256 + kq * 128
            return wt2[:, ei, c:c + 128]

        # ACT table preload: dependency-free exp at t=0 pulls the ~2.7us
        # ACT_TABLE_LOAD off the critical path of the first real exp.
        warm = spool.tile([128, 1], f32, tag="warm", name="warm")
        nc.vector.memset(warm, 0.0)

        # PE HAM warm-up weights
        wl = statics.tile([128, 128], f16, tag="wl", name="wl")
        wr = statics.tile([128, 512], f16, tag="wr", name="wr")
        nc.vector.memset(wl, 0.0)
        nc.vector.memset(wr, 0.0)

        # Input loads. A single HWDGE ring only sustains ~210 GB/s on HBM
        # reads, so the fill-critical wt0+x (1.9MB) is split across BOTH
        # rings (Sync + Scalar, ~350 GB/s combined). Each ring drains its
        # entries FIFO, so the remaining weights (needed only ~17us in)
        # sit strictly BEHIND x on the Scalar ring and cannot round-robin
        # bandwidth away from it.
        nc.sync.dma_start(wt0, wA_src)
        nc.scalar.dma_start(xt[:, 0:2, :], xP_src[:, 0:2, :])
        nc.sync.dma_start(xt[:, 2:4, :], xP_src[:, 2:4, :])
        nc.scalar.dma_start(xt[:, 4:6, :], xP_src[:, 4:6, :])
        nc.scalar.dma_start(wt1, wB1_src)
        nc.scalar.dma_start(wt2, wB2_src)

        nc.scalar.activation(warm, warm, Exp)

        # PE HAM warm-up: dummy matmuls keep the PE busy (and its clock
        # gate at 2.4 GHz) until the first x chunk lands; a >3.4us PE-idle
        # window would re-throttle it to 1.2 GHz and double the fill
        # projection's matmul time. Heavily deprioritized so the scheduler
        # only slots them where the PE would otherwise idle — they must
        # never delay a real matmul. Parks in a "ps" slot.
        pw = psum.tile([128, SPAN], f32, tag="ps", name="pw")
        for _ in range(9):
            nc.tensor.matmul(pw[:, 0:512], lhsT=wl, rhs=wr, start=True,
                             stop=True, skip_group_check=True)

        QUARTERS = [(0, 0), (0, 1), (1, 0), (1, 1)]  # K halves first

        # f-tile 0 projection, e-tile outer: the K0/K1/Q0 quarter
        # accumulations proceed as each x chunk lands instead of waiting
        # for all of x. K quarters borrow the (idle) score slots so their
        # casts free them for the first spans; Q0 sits in a proj slot.
        # Q1 (first needed by slice 4, several spans in) is deferred into
        # f-tile 0's interleave stream entirely.
        FILL_Q = [(0, 0, "ps"), (0, 1, "ps"), (1, 0, "pp")]
        f0_tiles = []
        for qi, (kq, half, tag) in enumerate(FILL_Q):
            shape = [128, SPAN] if tag == "ps" else [128, 512]
            f0_tiles.append(psum.tile(shape, f32, tag=tag, name=f"f0q{qi}"))

        def f0_mm(qi, ei):
            kq, half, _ = FILL_Q[qi]
            nc.tensor.matmul(
                f0_tiles[qi][:, 0:512],
                lhsT=w_block(0, kq, ei),
                rhs=xt[:, ei, half * 512:(half + 1) * 512],
                start=(ei == 0),
                stop=(ei == ET - 1),
                skip_group_check=True,
            )

        for ei in range(4):
            for qi in range(3):
                f0_mm(qi, ei)
        # last x chunk: quarter-major so K0 finishes first and its cast
        # overlaps the remaining matmuls
        for qi in range(3):
            for ei in (4, 5):
                f0_mm(qi, ei)
        # Casts needed before the first span: K both halves + the first
        # 256 Q columns (the first span's lhsT only reads qb0/qb1). Split
        # across the otherwise-idle ACT engine and DVE.
        nc.scalar.copy(qkt[:, 0, 0, 0:512], f0_tiles[0][:, 0:512])
        nc.vector.tensor_copy(qkt[:, 0, 0, 512:1024], f0_tiles[1][:, 0:512])
        nc.vector.tensor_copy(qkt[:, 0, 1, 0:256], f0_tiles[2][:, 0:256])
        nc.vector.tensor_copy(qkt[:, 0, 1, 256:512], f0_tiles[2][:, 256:512])

        def f0_q1_steps():
            holder = []
            for ei in range(ET):
                def mm_step(ei=ei, holder=holder):
                    if ei == 0:
                        holder.append(psum.tile(
                            [128, 512], f32, tag="pp", name="f0q3"))
                    nc.tensor.matmul(
                        holder[0],
                        lhsT=w_block(0, 1, ei),
                        rhs=xt[:, ei, 512:1024],
                        start=(ei == 0),
                        stop=(ei == ET - 1),
                        skip_group_check=True,
                    )
                yield mm_step

            def cast_step(holder=holder):
                nc.vector.tensor_copy(qkt[:, 0, 1, 512:1024], holder[0])
            yield cast_step

        def proj_steps(fi):
            # The projection for f-tile fi as a list of single-instruction
            # closures (6 accumulating matmuls + 1 evacuation cast per
            # quarter). Threading these one or two at a time between score
            # spans keeps the PE from blocking the ACT-paced span stream
            # (accumulation groups don't need to be contiguous in the PE
            # program; they only own their PSUM bank).
            steps = []
            for kq, half in QUARTERS:
                holder = []
                for ei in range(ET):
                    def mm_step(kq=kq, half=half, ei=ei, holder=holder):
                        if ei == 0:
                            holder.append(psum.tile(
                                [128, 512], f32, tag="pp",
                                name=f"pp{fi}_{kq}_{half}",
                            ))
                        nc.tensor.matmul(
                            holder[0],
                            lhsT=w_block(fi, kq, ei),
                            rhs=xt[:, ei, half * 512:(half + 1) * 512],
                            start=(ei == 0),
                            stop=(ei == ET - 1),
                            skip_group_check=True,
                        )
                    steps.append(mm_step)

                def cast_step(kq=kq, half=half, holder=holder):
                    nc.vector.tensor_copy(
                        qkt[:, fi, kq, half * 512:(half + 1) * 512],
                        holder[0],
                    )
                steps.append(cast_step)
            return steps

        def score_mm(ps, off, fi, qb, hh, nh):
            lo = 64 * hh
            nc.tensor.matmul(
                ps[:, off:off + 512],
                lhsT=qkt[lo:lo + 64, fi, 1, qb * 128:(qb + 1) * 128],
                rhs=qkt[lo:lo + 64, fi, 0, nh * 512:(nh + 1) * 512],
                start=True,
                stop=True,
                tile_position=(lo, 0),
            )

        def emit_attn(fi, interleave):
            # Slot s of y holds qb s (identity layout). A-slices (ACT
            # accumulator, self-contained 1024-col exps) are spaced between
            # span-aligned G-runs; the G-region generates more DVE work per
            # ACT span than ACT consumes (identity sums + recip + casts), so
            # A-slices let the DVE queue drain.
            a_last = fi == FT - 1
            a_set = A_SLICES_LAST if a_last else A_SLICES
            segs = []
            run = []
            for s in range(8):
                if s in a_set:
                    if run:
                        segs.append(("G", run))
                        run = []
                    segs.append(("A", [s]))
                else:
                    run.append(s)
            if run:
                segs.append(("G", run))

            y = ypool.tile([128, 16 * N], f16, tag="y", name=f"y{fi}")
            sums = spool.tile([128, 16], f32, tag="sums", name=f"sm{fi}")
            rec = spool.tile([128, 16], f32, tag="rec", name=f"rc{fi}")
            pending = list(interleave)

            def pull(k):
                # deprioritized: the pulled proj matmuls must not crowd
                # ahead of the next span's score matmuls in the PE queue
                with tc.high_priority(-PULL_DEPRIO):
                    for fn in pending[:k]:
                        fn()
                del pending[:k]

            def tile_sum(tt):
                yt = y[:, tt * N:(tt + 1) * N]
                nc.vector.tensor_scalar(
                    yt, yt, 1.0, 0.0, mult, add, accum_out=sums[:, tt:tt + 1],
                )

            def finish_slice(s):
                pull(1)
                t0 = 2 * s
                nc.vector.reciprocal(rec[:, t0:t0 + 2], sums[:, t0:t0 + 2])
                split_dma = a_last and s == 7
                for tt in (t0, t0 + 1):
                    yt = y[:, tt * N:(tt + 1) * N]
                    if a_last:
                        # both engines per slice: the pair runs concurrently,
                        # shortening the drain after the last exp
                        eng = nc.gpsimd if tt == t0 else nc.vector
                    else:
                        eng = nc.vector if s in DVE_MULT_STEADY else nc.gpsimd
                    eng.tensor_scalar(yt, yt, rec[:, tt:tt + 1], OUT_SCALE,
                                      mult, mult)
                    if split_dma:
                        nc.sync.dma_start(
                            out_ap[fi * 8 + s][:, (tt - t0) * N:(tt - t0 + 1) * N],
                            yt,
                        )
                if not split_dma:
                    nc.sync.dma_start(
                        out_ap[fi * 8 + s], y[:, s * 2048:(s + 1) * 2048]
                    )

            # Pulled proj steps land in the PE queue between the emitting
            # span's matmuls and the NEXT span's; inside an A-slice they
            # would delay the following G-span's matmuls past its exp and
            # open an ACT gap, so A-slices don't pull (except f-tile 0,
            # which has more steps to thread).
            a_pull = 2 if fi == 0 else 0
            g_pull = 2 if fi == 0 else 3
            for kind, qbs in segs:
                if kind == "A":
                    s = qbs[0]
                    for tt in (2 * s, 2 * s + 1):
                        hh = tt % 2
                        ps = psum.tile([128, SPAN], f32, tag="ps",
                                       name=f"psA{fi}_{tt}")
                        for nh in range(2):
                            score_mm(ps, nh * 512, fi, s, hh, nh)
                        nc.scalar.activation(
                            y[:, tt * N:(tt + 1) * N], ps[:, 0:N], Exp,
                            scale=SCALE, accum_out=sums[:, tt:tt + 1],
                        )
                        pull(a_pull)
                    finish_slice(s)
                else:
                    run0 = qbs[0] * 2048
                    run1 = run0 + len(qbs) * 2048
                    done = run0
                    # lead with the 1024 remainder span (if any): it is
                    # tile-aligned, so its row-sum rides the free ACT
                    # accumulator instead of a 1.14us DVE pass -- relieving
                    # DVE right after the A-slice where its congestion
                    # otherwise delays the proj casts
                    rem = (run1 - run0) % SPAN
                    span_ls = ([rem] if rem else []) + \
                        [SPAN] * ((run1 - run0) // SPAN)
                    c0 = run0
                    for L in span_ls:
                        ps = psum.tile([128, SPAN], f32, tag="ps",
                                       name=f"psG{fi}_{c0}")
                        # order chunks so PE row groups (hh) alternate:
                        # adjacent matmuls then run concurrently in the
                        # 128x128 array.
                        chunks = []
                        for off in range(0, L, 512):
                            g = c0 + off
                            chunks.append(
                                (off, g // 2048, (g // 1024) % 2, (g // 512) % 2))
                        h0 = [c for c in chunks if c[2] == 0]
                        h1 = [c for c in chunks if c[2] == 1]
                        ordered = []
                        a, b = (h0, h1) if len(h0) >= len(h1) else (h1, h0)
                        for i in range(len(chunks)):
                            src = a if i % 2 == 0 else b
                            if not src:
                                src = a if a else b
                            ordered.append(src.pop(0))
                        for off, sg, hh, nh in ordered:
                            score_mm(ps, off, fi, sg, hh, nh)
                        acc = L == N and c0 % N == 0
                        nc.scalar.activation(
                            y[:, c0:c0 + L], ps[:, 0:L], Exp, scale=SCALE,
                            accum_out=(sums[:, c0 // N:c0 // N + 1]
                                       if acc else None),
                        )
                        pull(g_pull)
                        new_done = ((c0 + L) // N) * N
                        for tt in range(done // N, new_done // N):
                            if not (acc and tt == c0 // N):
                                tile_sum(tt)
                            if tt % 2 == 1:
                                finish_slice(tt // 2)
                        done = new_done
                        c0 += L
            pull(len(pending))

        # f-tile 0's projection ran during the fill (minus Q1); fi+1's
        # projection threads into fi's attn stream an instruction at a time.
        for fi in range(FT):
            interleave = proj_steps(fi + 1) if fi + 1 < FT else []
            if fi == 0:
                interleave = list(f0_q1_steps()) + interleave
            emit_attn(fi, interleave)

    nc.compile()
    return nc


def _prep_inputs(x, W_qkv):
    x = np.asarray(x, dtype=np.float32)
    W = np.asarray(W_qkv, dtype=np.float32)
    # per-fi W column blocks [K_fi (128) | Q_fi (128)], then packed
    # partition-major: w[p, ei, c] = wT[ei*128+p, c]
    wq = W[0:768].reshape(FT, 128, E)        # Q blocks per f-tile
    wk = W[768:1536].reshape(FT, 128, E)     # K blocks per f-tile
    wkq = np.stack([wk, wq], axis=1)         # [fi, kq, 128, e]
    wT = wkq.transpose(3, 0, 1, 2).reshape(E, 2 * 128 * FT)  # [e, cols]
    wP = wT.reshape(ET, 128, 2 * 128 * FT).transpose(1, 0, 2)  # [p, ei, cols]
    wA = np.ascontiguousarray(wP[:, :, 0:256].reshape(128, -1)).astype(np.float16)
    wB1 = np.ascontiguousarray(wP[:, :, 256:512].reshape(128, -1)).astype(np.float16)
    wB2 = np.ascontiguousarray(wP[:, :, 512:1536].reshape(128, -1)).astype(np.float16)
    in_maps = []
    for b in range(B):
        xT = x[b].T                           # [e, n]
        xP = np.ascontiguousarray(
            xT.reshape(ET, 128, N).transpose(1, 0, 2).reshape(128, -1)
        ).astype(np.float16)
        in_maps.append({"xP": xP, "wA": wA, "wB1": wB1, "wB2": wB2})
    return in_maps


def _postprocess(res):
    outs = []
    inv = np.float32(1.0 / OUT_SCALE)
    for r in res.results:
        buf = r["out"]            # [48, 128, 2048] fp16, = out*1024
        buf = np.asarray(buf)
        if buf.dtype != np.float16:
            buf = buf.view(np.float16)
        full = buf.reshape(FT, 8, 128, 2, N).transpose(0, 3, 1, 2, 4)
        full = full.reshape(H, N, N).astype(np.float32) * inv
        outs.append(full)
    return np.stack(outs, axis=0)


def _run(x, W_qkv, trace=False):
    if "nc" not in _cache:
        _cache["nc"] = _build()
    nc = _cache["nc"]
    in_maps = _prep_inputs(x, W_qkv)
    res = run_bass_kernel_spmd(nc, in_maps, core_ids=list(range(B)), trace=trace)
    return _postprocess(res), res


def kernel(x, W_qkv):
    return _run(x, W_qkv)[0]
